# revision 1
# baseline (speedup 1.0000x reference)
"""MultiHeadAttention Trainium2 Bass kernel, 8-core SPMD.

Problem: B=4, S=2048, EMBED=1024, HEADS=16, HEAD_DIM=64 (fp32).
Sharding: core c -> batch b=c//2, query-half h=c%2 (1024 query rows).
Each core computes its 1024 output rows end-to-end; no collectives.

Per-core dataflow (all layouts transposed: feature/kk dim on partitions):
  A1: KT = WkT.T @ XkT + bk              -> SBUF resident  (f32r)
  A2: V  = XvT.T @ WvT (natural [kk,d])  -> SBUF resident bf16, with a
      ones column per head (V_aug) so the PV matmul also produces the
      softmax denominator for free.
  B:  per head-pair p:
      Q-proj for the pair (QT tile stays in SBUF; scale 1/8 + bias
      folded into the PSUM evacuation), then per q-chunk qc(512) and
      kk-pair kkp:
        S.T[kk,q] = KT_h.T @ QT_h        (f32r, heads row-packed in PE)
        P = exp(S.T)                      (ACT, no max-subtraction --
                                           scores ~ N(0,1), safe)
        P *= (1-mask).T                   (DVE, bf16 2x mode)
        OT_h[65,512] += V_aug_h.T @ P     (bf16; row 64 = denominator)
      normalize: OT_h[0:64] * recip(OT_h[64]) -> ot_dram (f32r)
  C:  outT = WoT.T @ OT + (bo + Wo @ bv)  -> DRAM [1024,1024]
Host reassembles out[b, h*1024:(h+1)*1024, :] = outT.T per core.
"""
import numpy as np
import ml_dtypes

import concourse.bass as bass
import concourse.mybir as mybir
import concourse.tile as tile
from concourse import bacc
from concourse.bass_utils import run_bass_kernel_spmd

F32R = mybir.dt.float32r
F32 = mybir.dt.float32
BF16 = mybir.dt.bfloat16
Act = mybir.ActivationFunctionType
Alu = mybir.AluOpType

EMBED = 1024
HEADS = 16
DH = 64
SQ = 1024   # query rows per core
SK = 2048   # key rows per core
NF = 8      # feature tiles (1024/128)
NKT = 16    # kk tiles (2048/128)
N_CORES = 8

_STATE = {}


def build_nc():
    nc = bacc.Bacc("TRN2", target_bir_lowering=False)
    xqT = nc.dram_tensor("xqT", [EMBED, SQ], F32R, kind="ExternalInput")
    xkT = nc.dram_tensor("xkT", [EMBED, SK], F32R, kind="ExternalInput")
    xvT = nc.dram_tensor("xvT", [EMBED, SK], F32R, kind="ExternalInput")
    wqT = nc.dram_tensor("wqT", [EMBED, EMBED], F32R, kind="ExternalInput")
    wkT = nc.dram_tensor("wkT", [EMBED, EMBED], F32R, kind="ExternalInput")
    wvT = nc.dram_tensor("wvT", [EMBED, EMBED], F32R, kind="ExternalInput")
    woT = nc.dram_tensor("woT", [EMBED, EMBED], F32R, kind="ExternalInput")
    bq8 = nc.dram_tensor("bq8", [128, NF], F32, kind="ExternalInput")
    bk_l = nc.dram_tensor("bk_l", [128, NF], F32, kind="ExternalInput")
    bo2 = nc.dram_tensor("bo2", [128, NF], F32, kind="ExternalInput")
    notmT = nc.dram_tensor("notmT", [SK, SQ], BF16, kind="ExternalInput")
    outT = nc.dram_tensor("outT", [EMBED, SQ], F32, kind="ExternalOutput")
    ot_dram = nc.dram_tensor("ot_dram", [EMBED, SQ], F32R)

    xqT_r = xqT.rearrange("(t p) q -> p t q", p=128)
    xkT_r = xkT.rearrange("(t p) k -> p t k", p=128)
    xvT_r = xvT.rearrange("(t p) k -> p t k", p=128)
    wqT_r = wqT.rearrange("(t p) n -> p t n", p=128)
    wkT_r = wkT.rearrange("(t p) n -> p t n", p=128)
    wvT_r = wvT.rearrange("(t p) n -> p t n", p=128)
    woT_r = woT.rearrange("(t p) n -> p t n", p=128)
    notmT_r = notmT.rearrange("(t p) q -> p t q", p=128)
    ot_dram_r = ot_dram.rearrange("(t p) q -> p t q", p=128)

    with tile.TileContext(nc) as tc:
        # ---------- persistent + early-prefetch pools ----------
        with tc.tile_pool(name="persist", bufs=1) as pp, \
             tc.tile_pool(name="bias", bufs=1) as bp, \
             tc.tile_pool(name="xv", bufs=2) as xvpool, \
             tc.tile_pool(name="bwq", bufs=2) as wqpool:
            kt = pp.tile([128, NF, SK], F32R, name="kt")
            vaug = pp.tile([128, NKT, HEADS * 65], BF16, name="vaug")
            bq8_sb = bp.tile([128, NF], F32, name="bq8_sb")
            bk_sb = bp.tile([128, NF], F32, name="bk_sb")
            bo2_sb = bp.tile([128, NF], F32, name="bo2_sb")

            # ---------- phase A1: K projection -> kt ----------
            with tc.tile_pool(name="a1w", bufs=1) as wpool, \
                 tc.tile_pool(name="a1x", bufs=2) as xpool, \
                 tc.tile_pool(name="a1p", bufs=4, space="PSUM") as pspool:
                wk_sb = wpool.tile([128, NF, EMBED], F32R, name="wk_sb")
                for ck in range(4):
                    xk_sb = xpool.tile([128, NF, 512], F32R, name="xk_sb")
                    if ck == 0:
                        nc.sync.dma_start(out=xk_sb[:, 0:2, :],
                                          in_=xkT_r[:, 0:2, 0:512])
                        nc.sync.dma_start(out=wk_sb[:, :, 0:128],
                                          in_=wkT_r[:, :, 0:128])
                        nc.sync.dma_start(out=xk_sb[:, 2:4, :],
                                          in_=xkT_r[:, 2:4, 0:512])
                        nc.sync.dma_start(out=xk_sb[:, 4:8, :],
                                          in_=xkT_r[:, 4:8, 0:512])
                        nc.sync.dma_start(out=wk_sb[:, :, 128:256],
                                          in_=wkT_r[:, :, 128:256])
                        nc.sync.dma_start(out=bk_sb[:], in_=bk_l[:, :])
                        nc.sync.dma_start(out=bq8_sb[:], in_=bq8[:, :])
                        nc.sync.dma_start(out=bo2_sb[:], in_=bo2[:, :])
                        for c4 in range(1, 4):
                            nc.sync.dma_start(
                                out=wk_sb[:, :, c4 * 256:(c4 + 1) * 256],
                                in_=wkT_r[:, :, c4 * 256:(c4 + 1) * 256])
                    else:
                        nc.sync.dma_start(
                            out=xk_sb[:],
                            in_=xkT_r[:, :, ck * 512:(ck + 1) * 512])
                    for m in range(NF):
                        ps = pspool.tile([128, 512], F32, name="a1ps")
                        for fi in range(NF):
                            nc.tensor.matmul(
                                ps[:], wk_sb[:, fi, m * 128:(m + 1) * 128],
                                xk_sb[:, fi, :],
                                start=(fi == 0), stop=(fi == NF - 1))
                        nc.vector.tensor_scalar(
                            out=kt[:, m, ck * 512:(ck + 1) * 512],
                            in0=ps[:], scalar1=bk_sb[:, m:m + 1],
                            scalar2=None, op0=Alu.add)

            # ---------- phase A2: V projection -> vaug (bf16 + ones) ----------
            # n-outer: heads 0..7 (n=0) complete first so phase B's first
            # head-pairs can overlap with the n=1 half.
            vaug_r = vaug.rearrange("p k (h c) -> p k h c", c=65)
            nc.vector.memset(vaug_r[:, :, :, 64:65], 1.0)
            xq_ctx = tc.tile_pool(name="xq", bufs=1)
            xqpool = xq_ctx.__enter__()
            xq_sb = xqpool.tile([128, NF, SQ], F32R, name="xq_sb")
            nc.sync.dma_start(out=xq_sb[:, :, 0:512], in_=xqT_r[:, :, 0:512])
            nc.sync.dma_start(out=xq_sb[:, :, 512:1024],
                              in_=xqT_r[:, :, 512:1024])
            with tc.tile_pool(name="a2w", bufs=2) as wpool, \
                 tc.tile_pool(name="a2p", bufs=4, space="PSUM") as pspool:
                wv_sb = []
                for n in range(2):
                    t = wpool.tile([128, NF, 512], F32R, name="wv_sb")
                    nc.sync.dma_start(out=t[:],
                                      in_=wvT_r[:, :, n * 512:(n + 1) * 512])
                    wv_sb.append(t)
                for m in range(NKT):
                    xv_sb = xvpool.tile([128, NF, 128], F32R,
                                        name="xv_sb")
                    nc.sync.dma_start(
                        out=xv_sb[:],
                        in_=xvT_r[:, :, m * 128:(m + 1) * 128])
                    for n in range(2):
                        ps = pspool.tile([128, 512], F32, name="a2ps")
                        for fi in range(NF):
                            nc.tensor.matmul(
                                ps[:], xv_sb[:, fi, :],
                                wv_sb[n][:, fi, :],
                                start=(fi == 0), stop=(fi == NF - 1))
                        nc.vector.tensor_copy(
                            out=vaug_r[:, m, n * 8:(n + 1) * 8, 0:64],
                            in_=ps.rearrange("p (h c) -> p h c", c=64))

            # ---------- phase B: Q-proj + attention, per head pair ----------
            with tc.tile_pool(name="bnotm", bufs=1) as nmpool, \
                 tc.tile_pool(name="bqt", bufs=2) as qpool, \
                 tc.tile_pool(name="bpt", bufs=2) as ptpool, \
                 tc.tile_pool(name="bnrm", bufs=2) as npool, \
                 tc.tile_pool(name="bst", bufs=1, space="PSUM") as stpool, \
                 tc.tile_pool(name="bqp", bufs=2, space="PSUM") as qppool, \
                 tc.tile_pool(name="bot", bufs=1, space="PSUM") as otpool:
                wq_first = wqpool.tile([128, NF, 128], F32R, name="wq_sb",
                                       tag="wq_sb")
                nc.sync.dma_start(out=wq_first[:], in_=wqT_r[:, :, 0:128])
                notm = nmpool.tile([128, NKT, SQ], BF16, name="notm")
                for c4 in range(4):
                    nc.sync.dma_start(
                        out=notm[:, c4 * 4:(c4 + 1) * 4, :],
                        in_=notmT_r[:, c4 * 4:(c4 + 1) * 4, :])
                for p in range(8):  # head pairs
                    # Q projection for this pair -> qt_sb [128, 1024] f32r
                    if p == 0:
                        wq_sb = wq_first
                    else:
                        wq_sb = wqpool.tile([128, NF, 128], F32R,
                                            name="wq_sb", tag="wq_sb")
                        nc.sync.dma_start(
                            out=wq_sb[:],
                            in_=wqT_r[:, :, p * 128:(p + 1) * 128])
                    qt_sb = qpool.tile([128, SQ], F32R, name="qt_sb")
                    for qc in range(2):
                        qps = qppool.tile([128, 512], F32, name="qps")
                        for fi in range(NF):
                            nc.tensor.matmul(
                                qps[:], wq_sb[:, fi, :],
                                xq_sb[:, fi, qc * 512:(qc + 1) * 512],
                                start=(fi == 0), stop=(fi == NF - 1))
                        nc.vector.tensor_scalar(
                            out=qt_sb[:, qc * 512:(qc + 1) * 512], in0=qps[:],
                            scalar1=0.125, scalar2=bq8_sb[:, p:p + 1],
                            op0=Alu.mult, op1=Alu.add)
                    for qc in range(2):
                        otps = [otpool.tile([128, 512], F32,
                                            name=f"otps{j}", tag=f"otps{j}")
                                for j in range(2)]
                        for kkp in range(8):
                            sts = [stpool.tile([128, 1024], F32,
                                               name=f"stps{j}", tag=f"stps{j}")
                                   for j in range(2)]
                            # ST matmuls interleaved by head so adjacent
                            # PE ops target disjoint row groups (0,0)/(64,0)
                            # and run concurrently (MMs are strict FIFO --
                            # same-row-group neighbors serialize).
                            def st_mm(hh, j):
                                lo = hh * 64
                                kkt = 2 * kkp + j
                                nc.tensor.matmul(
                                    sts[hh][:, j * 512:(j + 1) * 512],
                                    kt[lo:lo + 64, p,
                                       kkt * 128:(kkt + 1) * 128],
                                    qt_sb[lo:lo + 64,
                                          qc * 512:(qc + 1) * 512],
                                    start=True, stop=True,
                                    tile_position=(lo, 0))
                            pts = []
                            st_mm(0, 0)
                            st_mm(1, 0)
                            st_mm(0, 1)
                            pt0 = ptpool.tile([128, 1024], BF16,
                                              name="pt0", tag="pt0")
                            nc.scalar.activation(pt0[:], sts[0][:], Act.Exp)
                            pts.append(pt0)
                            st_mm(1, 1)
                            pt1 = ptpool.tile([128, 1024], BF16,
                                              name="pt1", tag="pt1")
                            nc.scalar.activation(pt1[:], sts[1][:], Act.Exp)
                            pts.append(pt1)
                            for hh in range(2):
                                h = 2 * p + hh
                                for j in range(2):
                                    kkt = 2 * kkp + j
                                    nc.vector.tensor_tensor(
                                        out=pts[hh][:, j * 512:(j + 1) * 512],
                                        in0=pts[hh][:, j * 512:(j + 1) * 512],
                                        in1=notm[:, kkt,
                                                 qc * 512:(qc + 1) * 512],
                                        op=Alu.mult)
                                    nc.tensor.matmul(
                                        otps[hh][0:65, :],
                                        vaug_r[:, kkt, h, :],
                                        pts[hh][:, j * 512:(j + 1) * 512],
                                        start=(kkp == 0 and j == 0),
                                        stop=(kkp == 7 and j == 1))
                        for hh in range(2):
                            rec = npool.tile([1, 512], F32, name="rec",
                                             tag="rec")
                            nc.vector.reciprocal(rec[:], otps[hh][64:65, :])
                            recb = npool.tile([64, 512], F32, name="recb",
                                              tag="recb")
                            nc.gpsimd.partition_broadcast(recb[:], rec[:])
                            otstg = npool.tile([64, 512], F32R, name="otstg",
                                               tag="otstg")
                            nc.vector.tensor_tensor(
                                out=otstg[:],
                                in0=otps[hh][0:64, :], in1=recb[:],
                                op=Alu.mult)
                            nc.sync.dma_start(
                                out=ot_dram[p * 128 + hh * 64:
                                            p * 128 + hh * 64 + 64,
                                            qc * 512:(qc + 1) * 512],
                                in_=otstg[:])

            # ---------- phase C: output projection ----------
            # wo streams through the (still open) bwq pool so the first
            # blocks prefetch during phase B's tail.
            with tc.tile_pool(name="cot", bufs=1) as cotpool, \
                 tc.tile_pool(name="cs", bufs=3) as spool, \
                 tc.tile_pool(name="cp", bufs=4, space="PSUM") as pspool:
                ot_sb = []
                for qc in range(2):
                    t = cotpool.tile([128, NF, 512], F32R, name=f"ot_sb{qc}")
                    if qc == 0:
                        nc.sync.dma_start(out=t[:, 0:4, :],
                                          in_=ot_dram_r[:, 0:4, 0:512])
                        nc.sync.dma_start(out=t[:, 4:8, :],
                                          in_=ot_dram_r[:, 4:8, 0:512])
                    else:
                        nc.sync.dma_start(
                            out=t[:],
                            in_=ot_dram_r[:, :, qc * 512:(qc + 1) * 512])
                    ot_sb.append(t)
                for m in range(NF):
                    wo_sb = wqpool.tile([128, NF, 128], F32R, name="wo_sb",
                                        tag="wq_sb")
                    nc.sync.dma_start(
                        out=wo_sb[:],
                        in_=woT_r[:, :, m * 128:(m + 1) * 128])
                    for qc in range(2):
                        ps = pspool.tile([128, 512], F32, name="cps")
                        for fi in range(NF):
                            nc.tensor.matmul(
                                ps[:], wo_sb[:, fi, :],
                                ot_sb[qc][:, fi, :],
                                start=(fi == 0), stop=(fi == NF - 1))
                        stg = spool.tile([128, 512], F32, name="cstg")
                        nc.vector.tensor_scalar(
                            out=stg[:], in0=ps[:],
                            scalar1=bo2_sb[:, m:m + 1], scalar2=None,
                            op0=Alu.add)
                        nc.sync.dma_start(
                            out=outT[m * 128:(m + 1) * 128,
                                     qc * 512:(qc + 1) * 512],
                            in_=stg[:])
            xq_ctx.__exit__(None, None, None)
    nc.compile()
    return nc


def _get_nc():
    if "nc" not in _STATE:
        _STATE["nc"] = build_nc()
    return _STATE["nc"]


def kernel(query, key, value, mask, Wq, bq, Wk, bk, Wv, bv, Wo, bo):
    query = np.asarray(query, dtype=np.float32)
    key = np.asarray(key, dtype=np.float32)
    value = np.asarray(value, dtype=np.float32)
    mask = np.asarray(mask)
    Wq = np.asarray(Wq, dtype=np.float32)
    Wk = np.asarray(Wk, dtype=np.float32)
    Wv = np.asarray(Wv, dtype=np.float32)
    Wo = np.asarray(Wo, dtype=np.float32)
    bq = np.asarray(bq, dtype=np.float32)
    bk = np.asarray(bk, dtype=np.float32)
    bv = np.asarray(bv, dtype=np.float32)
    bo = np.asarray(bo, dtype=np.float32)

    wqT = np.ascontiguousarray(Wq.T)
    wkT = np.ascontiguousarray(Wk.T)
    wvT = np.ascontiguousarray(Wv.T)
    woT = np.ascontiguousarray(Wo.T)
    bq8 = np.ascontiguousarray((bq / 8.0).reshape(NF, 128).T)
    bk_l = np.ascontiguousarray(bk.reshape(NF, 128).T)
    bo2v = bo + Wo @ bv
    bo2 = np.ascontiguousarray(bo2v.reshape(NF, 128).T)

    in_maps = []
    for c in range(N_CORES):
        b, h = c // 2, c % 2
        rows = slice(h * SQ, (h + 1) * SQ)
        xqTc = np.ascontiguousarray(query[b, rows, :].T)
        xkTc = np.ascontiguousarray(key[b].T)
        xvTc = np.ascontiguousarray(value[b].T)
        notm = np.ascontiguousarray(
            (~mask[b, 0, rows, :]).T.astype(ml_dtypes.bfloat16))
        in_maps.append({
            "xqT": xqTc, "xkT": xkTc, "xvT": xvTc,
            "wqT": wqT, "wkT": wkT, "wvT": wvT, "woT": woT,
            "bq8": bq8, "bk_l": bk_l, "bo2": bo2,
            "notmT": notm,
        })

    nc = _get_nc()
    res = run_bass_kernel_spmd(nc, in_maps, core_ids=list(range(N_CORES)))
    out = np.empty((4, 2048, EMBED), dtype=np.float32)
    for c in range(N_CORES):
        b, h = c // 2, c % 2
        out[b, h * SQ:(h + 1) * SQ, :] = res.results[c]["outT"].T
    return out



# revision 2
# speedup vs baseline: 1.2150x; 1.2150x over previous
"""MultiHeadAttention Trainium2 Bass kernel, 8-core SPMD.

Problem: B=4, S=2048, EMBED=1024, HEADS=16, HEAD_DIM=64 (fp32).

Sharding: core c -> batch b=c//2, head-half hh=c%2 (8 local heads,
Megatron-style tensor parallel). Each core computes the FULL 2048-query
attention for its 8 heads plus the partial output projection through its
512 columns of Wo; the host sums the two partials per batch (the
row-parallel all-reduce done at unshard time). No duplicated K/V
projection work and no device collectives.

All matmuls run in bf16 (1 PE cycle/row at any tile size); PSUM
accumulation stays f32. Per-core PE work: Q/K/V proj 3x27.3us +
attention 2x109.2us + O proj 27.3us = 327.6us, vs 398.8us for the
(batch, query-half) baseline -- the K/V duplication is gone.

Structure: a fused software pipeline over head pairs p=0..3. Round p
runs attention for pair p (ACT-heavy: exp) with the K/Q projections for
pair p+1 (PE-only) interleaved into the inner loop, so the PE fills the
gaps where exp is the per-iteration critical path. The V projection for
all 8 heads streams into round 0 the same way; the output projection for
q-chunk qc streams into round 3's chunk qc+1.

Per-core dataflow (feature/contraction dim on partitions):
  kt_p [128=2x64 dh, 2048 kk] = (Wk_p.T @ Xk + bk_p)/8, bf16
  qt_p [128, 2048 q]          = Wq_p.T @ Xq + bq_p, bf16
  vaug [128 kk, kkt, 8h x 65] = V bf16 + ones column per head (PV then
                                yields the softmax denominator free)
  per (qc 512q, kkp 2x128kk): S.T[kk,q] = kt_h.T @ qt_h (PE quadrants),
    P = exp(S.T) (ACT) -> bf16, P *= notm (DVE), OT[65,512] += Vaug.T @ P
  normalize: OT[0:64] * recip(OT[64]) -> ot_sb bf16 (stays in SBUF)
  outT_partial = Wo_l.T @ OT + bo2   (bo2 = Wo_l@bv_l + bo on core 0)
Host: out[b] = (outT[2b] + outT[2b+1]).T
"""
import numpy as np
import ml_dtypes

import concourse.bass as bass
import concourse.mybir as mybir
import concourse.tile as tile
from concourse import bacc
from concourse.bass_utils import run_bass_kernel_spmd

F32 = mybir.dt.float32
BF16 = mybir.dt.bfloat16
Act = mybir.ActivationFunctionType
Alu = mybir.AluOpType

EMBED = 1024
HEADS = 16
LH = 8        # local heads per core
PAIRS = 4     # local head pairs
DH = 64
SQ = 2048     # query rows (full)
SK = 2048     # key rows (full)
NF = 8        # contraction feature tiles (1024/128)
KKT = 16      # kk tiles of 128
QC = 4        # q chunks of 512
KKP = 8       # kk super-tiles of 256 (2 j x 128)
N_CORES = 8

_STATE = {}


def build_nc():
    nc = bacc.Bacc("TRN2", target_bir_lowering=False)
    xqT = nc.dram_tensor("xqT", [EMBED, SQ], BF16, kind="ExternalInput")
    xkT = nc.dram_tensor("xkT", [EMBED, SK], BF16, kind="ExternalInput")
    xvT = nc.dram_tensor("xvT", [EMBED, SK], BF16, kind="ExternalInput")
    wqT = nc.dram_tensor("wqT", [EMBED, 512], BF16, kind="ExternalInput")
    wkT = nc.dram_tensor("wkT", [EMBED, 512], BF16, kind="ExternalInput")
    wvT = nc.dram_tensor("wvT", [EMBED, 512], BF16, kind="ExternalInput")
    woT = nc.dram_tensor("woT", [512, EMBED], BF16, kind="ExternalInput")
    bq_l = nc.dram_tensor("bq_l", [128, PAIRS], F32, kind="ExternalInput")
    bk8_l = nc.dram_tensor("bk8_l", [128, PAIRS], F32, kind="ExternalInput")
    bo2_l = nc.dram_tensor("bo2_l", [128, NF], F32, kind="ExternalInput")
    notmT = nc.dram_tensor("notmT", [SK, SQ], BF16, kind="ExternalInput")
    outT = nc.dram_tensor("outT", [EMBED, SQ], F32, kind="ExternalOutput")

    xqT_r = xqT.rearrange("(t p) q -> p t q", p=128)
    xkT_r = xkT.rearrange("(t p) k -> p t k", p=128)
    xvT_r = xvT.rearrange("(t p) k -> p t k", p=128)
    wqT_r = wqT.rearrange("(t p) n -> p t n", p=128)
    wkT_r = wkT.rearrange("(t p) n -> p t n", p=128)
    wvT_r = wvT.rearrange("(t p) n -> p t n", p=128)
    woT_r = woT.rearrange("(t p) n -> p t n", p=128)
    notmT_r = notmT.rearrange("(t p) q -> p t q", p=128)

    with tile.TileContext(nc) as tc:
        with tc.tile_pool(name="bias", bufs=1) as bp, \
             tc.tile_pool(name="persist", bufs=1) as pp, \
             tc.tile_pool(name="ktqt", bufs=2) as kq, \
             tc.tile_pool(name="xq", bufs=2) as xqp, \
             tc.tile_pool(name="wkq", bufs=2) as wkqp, \
             tc.tile_pool(name="pt", bufs=2) as ptp, \
             tc.tile_pool(name="nrm", bufs=2) as nrmp, \
             tc.tile_pool(name="bst", bufs=1, space="PSUM") as stpool, \
             tc.tile_pool(name="bot", bufs=1, space="PSUM") as otpool, \
             tc.tile_pool(name="proj", bufs=2, space="PSUM") as projp:
            bq_sb = bp.tile([128, PAIRS], F32, name="bq_sb")
            bk8_sb = bp.tile([128, PAIRS], F32, name="bk8_sb")
            bo2_sb = bp.tile([128, NF], F32, name="bo2_sb")
            notm = pp.tile([128, KKT, SQ], BF16, name="notm")
            vaug = pp.tile([128, KKT, LH * 65], BF16, name="vaug")
            vaug_r = vaug.rearrange("p k (h c) -> p k h c", c=65)
            ot_sb = pp.tile([128, PAIRS, SQ], BF16, name="ot_sb")

            kt_t = {}
            qt_t = {}

            def alloc_ktqt(p):
                kt_t[p] = kq.tile([128, SK], BF16, name=f"kt{p}", tag="kt")
                qt_t[p] = kq.tile([128, SQ], BF16, name=f"qt{p}", tag="qt")

            # ---------- instruction-level group builders ----------
            # Each builder returns a list of zero-arg callables; one callable
            # emits one (or a tightly-coupled few) instruction(s). The
            # attention loop pulls these between its own PE instructions so
            # projection matmuls fill PE stalls where exp is critical.

            def kproj_group(wk_sb, p, kkc, xk_sb):
                ps = projp.tile([128, 512], F32, name="kps", tag="proj")
                ops = []
                for fi in range(NF):
                    ops.append(lambda fi=fi, ps=ps: nc.tensor.matmul(
                        ps[:], wk_sb[:, fi, :],
                        xk_sb[:, fi, kkc * 512:(kkc + 1) * 512],
                        start=(fi == 0), stop=(fi == NF - 1)))
                ops.append(lambda ps=ps: nc.vector.tensor_scalar(
                    out=kt_t[p][:, kkc * 512:(kkc + 1) * 512], in0=ps[:],
                    scalar1=0.125, scalar2=bk8_sb[:, p:p + 1],
                    op0=Alu.mult, op1=Alu.add))
                return ops

            def qproj_group(wq_sb, p, qc, xq_ch):
                ps = projp.tile([128, 512], F32, name="qps", tag="proj")
                ops = []
                for fi in range(NF):
                    ops.append(lambda fi=fi, ps=ps: nc.tensor.matmul(
                        ps[:], wq_sb[:, fi, :], xq_ch[:, fi, :],
                        start=(fi == 0), stop=(fi == NF - 1)))
                ops.append(lambda ps=ps: nc.vector.tensor_scalar(
                    out=qt_t[p][:, qc * 512:(qc + 1) * 512], in0=ps[:],
                    scalar1=bq_sb[:, p:p + 1], scalar2=None, op0=Alu.add))
                return ops

            def vproj_group(wv_sb, kkt, xv_ch, sub):
                # xv_ch covers kk tiles [2c, 2c+1]; sub selects which.
                ps = projp.tile([128, 512], F32, name="vps", tag="proj")
                ops = []
                for fi in range(NF):
                    ops.append(lambda fi=fi, ps=ps: nc.tensor.matmul(
                        ps[:], xv_ch[:, fi, sub * 128:(sub + 1) * 128],
                        wv_sb[:, fi, :],
                        start=(fi == 0), stop=(fi == NF - 1)))
                ops.append(lambda ps=ps: nc.vector.tensor_copy(
                    out=vaug_r[:, kkt, :, 0:64],
                    in_=ps.rearrange("p (h c) -> p h c", c=64)))
                return ops

            def oproj_group(wo_sb, m, qc, cstage):
                ps = projp.tile([128, 512], F32, name="ops", tag="proj")
                ops = []
                for fp in range(PAIRS):
                    ops.append(lambda fp=fp, ps=ps: nc.tensor.matmul(
                        ps[:], wo_sb[:, fp, m * 128:(m + 1) * 128],
                        ot_sb[:, fp, qc * 512:(qc + 1) * 512],
                        start=(fp == 0), stop=(fp == PAIRS - 1)))

                def evac(ps=ps):
                    stg = cstage.tile([128, 512], F32, name="cstg")
                    nc.vector.tensor_scalar(
                        out=stg[:], in0=ps[:], scalar1=bo2_sb[:, m:m + 1],
                        scalar2=None, op0=Alu.add)
                    nc.sync.dma_start(
                        out=outT[m * 128:(m + 1) * 128,
                                 qc * 512:(qc + 1) * 512],
                        in_=stg[:])
                ops.append(evac)
                return ops

            # ---------- attention inner iteration ----------
            def attn_iter(p, qc, kkp, otps, extra):
                for fn in extra:
                    fn()
                sts = [stpool.tile([128, 1024], F32, name=f"stps{j}",
                                   tag=f"stps{j}") for j in range(2)]

                def st_mm(hh, j):
                    lo = hh * 64
                    kkt = 2 * kkp + j
                    nc.tensor.matmul(
                        sts[hh][:, j * 512:(j + 1) * 512],
                        kt_t[p][lo:lo + 64, kkt * 128:(kkt + 1) * 128],
                        qt_t[p][lo:lo + 64, qc * 512:(qc + 1) * 512],
                        start=True, stop=True, tile_position=(lo, 0))

                pts = []
                st_mm(0, 0)
                st_mm(1, 0)
                st_mm(0, 1)
                pt0 = ptp.tile([128, 1024], BF16, name="pt0", tag="pt0")
                nc.scalar.activation(pt0[:], sts[0][:], Act.Exp)
                pts.append(pt0)
                st_mm(1, 1)
                pt1 = ptp.tile([128, 1024], BF16, name="pt1", tag="pt1")
                nc.scalar.activation(pt1[:], sts[1][:], Act.Exp)
                pts.append(pt1)
                for hh in range(2):
                    nc.vector.tensor_tensor(
                        out=pts[hh].rearrange("p (j q) -> p j q", q=512),
                        in0=pts[hh].rearrange("p (j q) -> p j q", q=512),
                        in1=notm[:, 2 * kkp:2 * kkp + 2,
                                 qc * 512:(qc + 1) * 512],
                        op=Alu.mult)
                for hh in range(2):
                    for j in range(2):
                        kkt = 2 * kkp + j
                        nc.tensor.matmul(
                            otps[hh][0:65, :],
                            vaug_r[:, kkt, 2 * p + hh, :],
                            pts[hh][:, j * 512:(j + 1) * 512],
                            start=(kkp == 0 and j == 0),
                            stop=(kkp == KKP - 1 and j == 1))

            def normalize(p, qc, otps):
                for hh in range(2):
                    rec = nrmp.tile([1, 512], F32, name="rec", tag="rec")
                    nc.vector.reciprocal(rec[:], otps[hh][64:65, :])
                    recb = nrmp.tile([64, 512], F32, name="recb", tag="recb")
                    nc.gpsimd.partition_broadcast(recb[:], rec[:])
                    nc.vector.tensor_tensor(
                        out=ot_sb[hh * 64:(hh + 1) * 64, p,
                                  qc * 512:(qc + 1) * 512],
                        in0=otps[hh][0:64, :], in1=recb[:], op=Alu.mult)

            # ---------- prologue ----------
            nc.sync.dma_start(out=bq_sb[:], in_=bq_l[:, :])
            nc.sync.dma_start(out=bk8_sb[:], in_=bk8_l[:, :])
            nc.sync.dma_start(out=bo2_sb[:], in_=bo2_l[:, :])
            nc.vector.memset(vaug_r[:, :, :, 64:65], 1.0)

            with tc.tile_pool(name="xk", bufs=1) as xkp:
                xk_sb = xkp.tile([128, NF, SK], BF16, name="xk_sb")
                wk_sb = wkqp.tile([128, NF, 128], BF16, name="wk0", tag="wk")
                nc.sync.dma_start(out=wk_sb[:], in_=wkT_r[:, :, 0:128])
                for kkc in range(4):
                    nc.sync.dma_start(
                        out=xk_sb[:, :, kkc * 512:(kkc + 1) * 512],
                        in_=xkT_r[:, :, kkc * 512:(kkc + 1) * 512])
                wq_sb = wkqp.tile([128, NF, 128], BF16, name="wq0", tag="wq")
                nc.sync.dma_start(out=wq_sb[:], in_=wqT_r[:, :, 0:128])

                with tc.tile_pool(name="xv", bufs=2) as xvp, \
                     tc.tile_pool(name="wv", bufs=1) as wvp:
                    wv_sb = wvp.tile([128, NF, 512], BF16, name="wv_sb")
                    nc.gpsimd.dma_start(out=wv_sb[:], in_=wvT_r[:, :, :])
                    xv_chunks = {}

                    def load_xv(c):
                        t = xvp.tile([128, NF, 256], BF16, name="xv_ch",
                                     tag="xv")
                        nc.gpsimd.dma_start(
                            out=t[:], in_=xvT_r[:, :, c * 256:(c + 1) * 256])
                        xv_chunks[c] = t

                    load_xv(0)
                    # notm chunks 0,1 early (round-0 masks), rest after xq.
                    for nchunk in range(2):
                        nc.gpsimd.dma_start(
                            out=notm[:, nchunk * 4:(nchunk + 1) * 4, :],
                            in_=notmT_r[:, nchunk * 4:(nchunk + 1) * 4, :])
                    xq_chunks = {}

                    def load_xq(p, qc):
                        t = xqp.tile([128, NF, 512], BF16, name="xq_ch",
                                     tag="xq")
                        nc.gpsimd.dma_start(
                            out=t[:], in_=xqT_r[:, :, qc * 512:(qc + 1) * 512])
                        xq_chunks[(p, qc)] = t

                    for qc in range(QC):
                        load_xq(0, qc)
                    load_xv(1)
                    for nchunk in range(2, 4):
                        nc.gpsimd.dma_start(
                            out=notm[:, nchunk * 4:(nchunk + 1) * 4, :],
                            in_=notmT_r[:, nchunk * 4:(nchunk + 1) * 4, :])

                    # prologue compute: K/Q proj for pair 0, V proj kkt 0,1
                    alloc_ktqt(0)
                    for kkc in range(4):
                        for fn in kproj_group(wk_sb, 0, kkc, xk_sb):
                            fn()
                    for qc in range(QC):
                        for fn in qproj_group(wq_sb, 0, qc,
                                              xq_chunks[(0, qc)]):
                            fn()
                    for kkt in range(2):
                        for fn in vproj_group(wv_sb, kkt, xv_chunks[0], kkt):
                            fn()

                    # ---------- rounds 0..2 ----------
                    for p in range(3):
                        # work to interleave into this round: K/Q proj for
                        # pair p+1 (+ V proj for kkt 2..15 in round 0).
                        slots = [[] for _ in range(QC * KKP)]
                        if p == 0:
                            for i in range(7):
                                for kkt in (2 * i + 2, 2 * i + 3):
                                    c = kkt // 2
                                    if c not in xv_chunks:
                                        load_xv(c)
                                    slots[i] += vproj_group(
                                        wv_sb, kkt, xv_chunks[c], kkt % 2)
                        alloc_ktqt(p + 1)
                        wk_n = wkqp.tile([128, NF, 128], BF16,
                                         name=f"wk{p+1}", tag="wk")
                        nc.sync.dma_start(
                            out=wk_n[:],
                            in_=wkT_r[:, :, (p + 1) * 128:(p + 2) * 128])
                        wq_n = wkqp.tile([128, NF, 128], BF16,
                                         name=f"wq{p+1}", tag="wq")
                        nc.sync.dma_start(
                            out=wq_n[:],
                            in_=wqT_r[:, :, (p + 1) * 128:(p + 2) * 128])
                        for qc in range(QC):
                            load_xq(p + 1, qc)
                        groups = [kproj_group(wk_n, p + 1, kkc, xk_sb)
                                  for kkc in range(4)]
                        groups += [qproj_group(wq_n, p + 1, qc,
                                               xq_chunks[(p + 1, qc)])
                                   for qc in range(QC)]
                        for gi, g in enumerate(groups):
                            slots[8 + 3 * gi] += g

                        for qc in range(QC):
                            otps = [otpool.tile([128, 512], F32,
                                                name=f"otps{h}",
                                                tag=f"otps{h}")
                                    for h in range(2)]
                            for kkp in range(KKP):
                                attn_iter(p, qc, kkp, otps,
                                          slots[qc * KKP + kkp])
                            normalize(p, qc, otps)

            # ---------- round 3 (+ interleaved output projection) ----------
            with tc.tile_pool(name="wo", bufs=1) as wop, \
                 tc.tile_pool(name="cs", bufs=3) as cstage:
                wo_sb = wop.tile([128, PAIRS, EMBED], BF16, name="wo_sb")
                nc.gpsimd.dma_start(out=wo_sb[:], in_=woT_r[:, :, :])
                p = 3
                slots = [[] for _ in range(QC * KKP)]
                for qcd in range(3):  # oproj for qc=qcd during chunk qcd+1
                    for m in range(NF):
                        slots[(qcd + 1) * KKP + m] += oproj_group(
                            wo_sb, m, qcd, cstage)
                for qc in range(QC):
                    otps = [otpool.tile([128, 512], F32, name=f"otps{h}",
                                        tag=f"otps{h}") for h in range(2)]
                    for kkp in range(KKP):
                        attn_iter(p, qc, kkp, otps, slots[qc * KKP + kkp])
                    normalize(p, qc, otps)
                for m in range(NF):
                    for fn in oproj_group(wo_sb, m, 3, cstage):
                        fn()
    nc.compile()
    return nc


def _get_nc():
    if "nc" not in _STATE:
        _STATE["nc"] = build_nc()
    return _STATE["nc"]


BF = ml_dtypes.bfloat16


def kernel(query, key, value, mask, Wq, bq, Wk, bk, Wv, bv, Wo, bo):
    query = np.asarray(query, dtype=np.float32)
    key = np.asarray(key, dtype=np.float32)
    value = np.asarray(value, dtype=np.float32)
    mask = np.asarray(mask)
    Wq = np.asarray(Wq, dtype=np.float32)
    Wk = np.asarray(Wk, dtype=np.float32)
    Wv = np.asarray(Wv, dtype=np.float32)
    Wo = np.asarray(Wo, dtype=np.float32)
    bq = np.asarray(bq, dtype=np.float32)
    bk = np.asarray(bk, dtype=np.float32)
    bv = np.asarray(bv, dtype=np.float32)
    bo = np.asarray(bo, dtype=np.float32)

    wqT = Wq.T  # [in 1024, out 1024]
    wkT = Wk.T
    wvT = Wv.T
    woT = Wo.T  # [in 1024 (concat heads), out 1024]

    xT = {}
    nmT = {}
    for b in range(4):
        xT[("q", b)] = np.ascontiguousarray(query[b].T.astype(BF))
        xT[("k", b)] = np.ascontiguousarray(key[b].T.astype(BF))
        xT[("v", b)] = np.ascontiguousarray(value[b].T.astype(BF))
        nmT[b] = np.ascontiguousarray((~mask[b, 0]).T.astype(BF))

    half = {}
    for hh in range(2):
        cols = slice(hh * 512, (hh + 1) * 512)
        bo2 = woT[cols, :].T @ bv[cols]
        if hh == 0:
            bo2 = bo2 + bo
        half[hh] = {
            "wqT": np.ascontiguousarray(wqT[:, cols].astype(BF)),
            "wkT": np.ascontiguousarray(wkT[:, cols].astype(BF)),
            "wvT": np.ascontiguousarray(wvT[:, cols].astype(BF)),
            "woT": np.ascontiguousarray(woT[cols, :].astype(BF)),
            "bq_l": np.ascontiguousarray(
                bq[cols].reshape(PAIRS, 128).T.astype(np.float32)),
            "bk8_l": np.ascontiguousarray(
                (bk[cols] / 8.0).reshape(PAIRS, 128).T.astype(np.float32)),
            "bo2_l": np.ascontiguousarray(
                bo2.reshape(NF, 128).T.astype(np.float32)),
        }

    in_maps = []
    for c in range(N_CORES):
        b, hh = c // 2, c % 2
        m = {"xqT": xT[("q", b)], "xkT": xT[("k", b)], "xvT": xT[("v", b)],
             "notmT": nmT[b]}
        m.update(half[hh])
        in_maps.append(m)

    nc = _get_nc()
    res = run_bass_kernel_spmd(nc, in_maps, core_ids=list(range(N_CORES)))
    out = np.empty((4, 2048, EMBED), dtype=np.float32)
    for b in range(4):
        acc = res.results[2 * b]["outT"] + res.results[2 * b + 1]["outT"]
        out[b] = acc.T
    return out
